# revision 1
# baseline (speedup 1.0000x reference)
"""GPTQ 4-bit quantized linear (column-parallel over 8 NeuronCores), v2.

y = x @ dequant(qweight, scales, zeros).T + bias with byte-packed 4-bit pairs.

Per core (out_features 11008 -> 8 x 1376, padded 1408):
  * Host repacks nibbles into wq[j, o] (j = i mod 2048, lo nibble -> group
    j//128, hi nibble -> group j//128 + 16), exactly as a bit permutation.
  * Host precomputes corrb[o, b] = bias[o] - sum_g (s*z)[o,g] * X[g,b]
    (X[g,b] = sum_{i in g} x[b,i]) so no on-device zero-correction is needed.
  * k-tiles are unpacked via one of two paths:
      f8: host pre-unpacks nibbles to exact fp8e4 (DMA 2x bytes, zero engine
          work); matmuls use an exact x = xh8 + xl8 fp8 split (2 MMs/slot).
      packed: DVE masks (u16 bitwise, 4x mode) then u8->f16 casts on
          ACT / GpSimd / DVE per a static schedule (hi tile holds 16*hi; the
          matching scale columns are pre-divided by 16 on host).
  * Per-group partial dot products D[o, c, b] accumulate in PSUM with the
    group->column map c = (2j lo, 2j+1 hi) so any k-range is a contiguous
    column block; the scale-and-reduce fix runs in two pipelined pieces
    (A: k0-11 overlaps the k stream, B: k12-15 is a small tail split
    between DVE and GpSimd).
  * Fix per bank/piece: tmp[o,b,g] = D * s (broadcast TT), reduce over g,
    add into corrb accumulator; the final piece emits f16.
"""

import numpy as np
import ml_dtypes

import concourse.bacc as bacc
import concourse.mybir as mybir
import concourse.tile as tile
from concourse.bass_utils import run_bass_kernel_spmd

dt = mybir.dt
Alu = mybir.AluOpType

B = 8
I = 4096
O = 11008
NCORES = 8
OSH = O // NCORES            # 1376
OT = 11
OPAD = OT * 128              # 1408 (fix/output padding)
OPW = OSH                    # 1376: weight tensors are exact-width
NG = 32
NJ = 16                      # packed k-tiles (2048 / 128)
NBANK = 6                    # PSUM banks for D (2 o-tiles per bank)

# k-tile routes: "f8" (host-unpacked fp8) or two cast engines for lo/hi
# from {a: ACT, p: Pool, v: DVE}. f8 tiles and packed tiles must both come
# in consecutive pairs (k even -> k, k+1 same class) for paired DMAs.
ROUTES = ["aa", "pv", "f8", "f8", "aa", "pv", "f8", "f8",
          "ap", "ap", "f8", "f8", "f8", "f8", "f8", "f8"]
PIECES = [8, 12, 16]         # fix pieces by completer count (cumulative)
WARM = 0


def _completion_order():
    # crude static completion model: packed tiles finish when their last cast
    # clears its engine chain; f8 tiles when their (paired) DMA lands.
    act_t, pool_t = 3.3, 3.4
    n_pk_pairs = sum(1 for k in range(NJ) if ROUTES[k] != "f8") / 2.0
    dma_t = 1.6
    f8_dma = 1.6 + n_pk_pairs * 1.0
    done = {}
    for k in range(NJ):
        r = ROUTES[k]
        if r == "f8":
            if k % 2 == 0:
                f8_dma += 2.0
            done[k] = f8_dma + 1.0
        else:
            if k % 2 == 0:
                dma_t += 1.0
            t = dma_t + 0.9
            for e in r:
                if e == "a":
                    act_t = max(act_t, t) + 1.36
                    t = act_t
                elif e == "p":
                    pool_t = max(pool_t, t) + 2.05
                    t = pool_t
                else:
                    t = t + 0.8
            done[k] = t
    return sorted(range(NJ), key=lambda k: done[k])


_ORDER = None


def colmap(j):
    global _ORDER
    if _ORDER is None:
        _ORDER = _completion_order()
    pos = _ORDER.index(j)
    return (2 * pos, 2 * pos + 1)


_nc_cache = None


def _pairs(ks):
    out = []
    i = 0
    while i < len(ks):
        if i + 1 < len(ks) and ks[i + 1] == ks[i] + 1:
            out.append((ks[i], 2))
            i += 2
        else:
            out.append((ks[i], 1))
            i += 1
    return out


def _build_nc():
    nc = bacc.Bacc("TRN2", target_bir_lowering=False)

    packed_ks = [k for k in range(NJ) if ROUTES[k] != "f8"]
    f8_ks = [k for k in range(NJ) if ROUTES[k] == "f8"]
    pk_chunks = _pairs(packed_ks)
    f8_chunks = _pairs(f8_ks)

    wq_d = {k0: nc.dram_tensor(f"wq{k0}", [n, 128, OPW], dt.uint8,
                               kind="ExternalInput") for k0, n in pk_chunks}
    f8_d = {k0: nc.dram_tensor(f"f8_{k0}", [128, n * 2 * OPW], dt.float8e4,
                               kind="ExternalInput") for k0, n in f8_chunks}
    cblob_d = nc.dram_tensor("cblob", [128, 2080], dt.uint8, kind="ExternalInput")
    out = nc.dram_tensor("out", [OPAD, B], dt.float16, kind="ExternalOutput")

    with tile.TileContext(nc) as tc:
        with (
            tc.tile_pool(name="const", bufs=1) as constp,
            tc.tile_pool(name="wqp", bufs=3) as wqp,
            tc.tile_pool(name="f8p", bufs=3) as f8p,
            tc.tile_pool(name="nibp", bufs=8) as nibp,
            tc.tile_pool(name="fixp", bufs=1) as fixp,
            tc.tile_pool(name="dpsp", bufs=1, space="PSUM") as dpsp,
        ):
            # ---- all constants in one DMA (unblock matmuls + fix early)
            cblob = constp.tile([128, 2080], dt.uint8)
            nc.sync.dma_start(cblob[:], cblob_d[:])
            xt_sb = cblob[:, 0:512].bitcast(dt.float16)
            x8_sb = cblob[:, 512:1024].bitcast(dt.float8e4)
            sfix_sb = cblob[:, 1024:1728].bitcast(dt.float16)
            corrb_sb = cblob[:, 1728:2080].bitcast(dt.float32)

            # ---- weight DMAs in k order
            wq_sb = {}
            f8_sb = {}
            for k0, n in pk_chunks:
                wq_sb[k0] = wqp.tile([128, n, OPW], dt.uint8, name=f"wq{k0}")
                nc.sync.dma_start(wq_sb[k0][:],
                                  wq_d[k0][:].rearrange("n p o -> p n o"))
            for k0, n in f8_chunks:
                f8_sb[k0] = f8p.tile([128, n * 2 * OPW], dt.float8e4,
                                     name=f"f8_{k0}")
                nc.sync.dma_start(f8_sb[k0][:], f8_d[k0][:])

            if WARM:
                junk = constp.tile([128, 128], dt.float16)
                nc.gpsimd.memset(junk[:], 0.0)
                warm_ps = dpsp.tile([128, 128], dt.float32, name="dwarm",
                                    tag="warm")
                for _ in range(WARM):
                    nc.tensor.matmul(warm_ps[:], junk[:], junk[:],
                                     start=True, stop=True)

            # piece 0: 16 cols, 3 tiles of 4/4/3 o-tiles;
            # pieces 1,2: 8 cols, 2 tiles of 6/5 o-tiles.
            d_ps = [
                [
                    dpsp.tile([128, 4 if j < 2 else 3, 16 * B], dt.float32,
                              name=f"d0_{j}", tag=f"d0_{j}")
                    for j in range(3)
                ],
                [
                    dpsp.tile([128, 6 if j == 0 else 5, 8 * B], dt.float32,
                              name=f"d1_{j}", tag=f"d1_{j}")
                    for j in range(2)
                ],
                [
                    dpsp.tile([128, 6 if j == 0 else 5, 8 * B], dt.float32,
                              name=f"d2_{j}", tag=f"d2_{j}")
                    for j in range(2)
                ],
            ]

            def dview(pi, t):
                if pi == 0:
                    return d_ps[0][t // 4][:, t % 4]
                return d_ps[pi][0 if t < 6 else 1][:, t if t < 6 else t - 6]

            xh8 = x8_sb[:, : NG * B]
            xl8 = x8_sb[:, NG * B:]

            def mms(k, lo_ap, hi_ap, fp8):
                clo, chi = colmap(k)
                glo, ghi = k, k + 16
                pi = next(i for i, pc in enumerate(PIECES) if clo < 2 * pc)
                base = 0 if pi == 0 else 2 * PIECES[pi - 1]
                clo -= base
                chi -= base
                for t in range(OT):
                    w = 128 if t < OT - 1 else OPW - 128 * (OT - 1)
                    dv = dview(pi, t)[: w]
                    ls = lo_ap[:, t * 128:t * 128 + w]
                    hs = hi_ap[:, t * 128:t * 128 + w]
                    if fp8:
                        nc.tensor.matmul(dv[:, clo * B:(clo + 1) * B], ls,
                                         xh8[:, glo * B:(glo + 1) * B],
                                         start=True, stop=False)
                        nc.tensor.matmul(dv[:, clo * B:(clo + 1) * B], ls,
                                         xl8[:, glo * B:(glo + 1) * B],
                                         start=False, stop=True)
                        nc.tensor.matmul(dv[:, chi * B:(chi + 1) * B], hs,
                                         xh8[:, ghi * B:(ghi + 1) * B],
                                         start=True, stop=False)
                        nc.tensor.matmul(dv[:, chi * B:(chi + 1) * B], hs,
                                         xl8[:, ghi * B:(ghi + 1) * B],
                                         start=False, stop=True)
                    else:
                        nc.tensor.matmul(dv[:, clo * B:(clo + 1) * B], ls,
                                         xt_sb[:, glo * B:(glo + 1) * B],
                                         start=True, stop=True)
                        nc.tensor.matmul(dv[:, chi * B:(chi + 1) * B], hs,
                                         xt_sb[:, ghi * B:(ghi + 1) * B],
                                         start=True, stop=True)

            cast_eng = {"p": nc.gpsimd, "v": nc.vector}
            fix_eng = {"v": nc.vector, "p": nc.gpsimd}

            uA = fixp.tile([128, NBANK * 2 * B], dt.float32, name="uA")
            yt = fixp.tile([128, OT * B], dt.float16, name="yt")

            def fix_piece(j, pi):
                c0 = 0 if pi == 0 else 2 * PIECES[pi - 1]
                ncols = 2 * (PIECES[pi] - (0 if pi == 0 else PIECES[pi - 1]))
                if pi == 0:
                    t0, nt = 4 * j, (4 if j < 2 else 3)
                else:
                    t0, nt = (0, 6) if j == 0 else (6, 5)
                dv = d_ps[pi][j][:].rearrange("p o (c b) -> p o c b", b=B)
                sf = sfix_sb.rearrange("p (t c) -> p t c", c=NG)[
                    :, t0:t0 + nt, c0:c0 + ncols]
                tmp = fixp.tile([128, nt, B, ncols], dt.float16,
                                tag=f"tmp{j}_{pi}", name=f"tmp{j}_{pi}")
                nc.vector.tensor_tensor(
                    tmp[:].transpose([0, 1, 3, 2]), dv,
                    sf.unsqueeze(3).broadcast_to([128, nt, ncols, B]), Alu.mult)
                red = fixp.tile([128, nt * B], dt.float16, tag=f"red{j}_{pi}",
                                name=f"red{j}_{pi}")
                with nc.allow_low_precision(reason="f16 partial sums"):
                    nc.vector.tensor_reduce(
                        red[:].rearrange("p (o b) -> p o b", b=B), tmp[:],
                        axis=mybir.AxisListType.X, op=Alu.add)
                sl = slice(t0 * B, (t0 + nt) * B)
                if pi == 0:
                    nc.vector.tensor_tensor(uA[:, sl], corrb_sb[:, sl], red[:],
                                            Alu.add)
                elif pi == 1:
                    nc.vector.tensor_tensor(uA[:, sl], uA[:, sl], red[:], Alu.add)
                else:
                    nc.vector.tensor_tensor(yt[:, sl], uA[:, sl], red[:], Alu.add)

            def unpack_and_mm(k):
                r = ROUTES[k]
                if r == "f8":
                    for k0, n in f8_chunks:
                        if k0 <= k < k0 + n:
                            off = (k - k0) * 2 * OPW
                            f8t = f8_sb[k0][:, off:off + 2 * OPW]
                    mms(k, f8t[:, :OPW], f8t[:, OPW:], True)
                    return
                for k0, n in pk_chunks:
                    if k0 <= k < k0 + n:
                        src = wq_sb[k0][:, k - k0]
                wq16 = src.bitcast(dt.uint16)
                lo8 = nibp.tile([128, OPW], dt.uint8, tag="lo8", name=f"lo8_{k}")
                nc.vector.tensor_scalar(lo8[:].bitcast(dt.uint16), wq16,
                                        0x0F0F, None, op0=Alu.bitwise_and)
                hi8 = nibp.tile([128, OPW], dt.uint8, tag="hi8", name=f"hi8_{k}")
                nc.vector.tensor_scalar(hi8[:].bitcast(dt.uint16), wq16,
                                        0xF0F0, None, op0=Alu.bitwise_and)
                lo = nibp.tile([128, OPW], dt.float16, tag="lo", name=f"lo_{k}")
                if r[0] == "a":
                    nc.scalar.copy(lo[:], lo8[:])
                else:
                    cast_eng[r[0]].tensor_copy(lo[:], lo8[:])
                hi = nibp.tile([128, OPW], dt.float16, tag="hi", name=f"hi_{k}")
                if r[1] == "a":
                    nc.scalar.copy(hi[:], hi8[:])
                else:
                    cast_eng[r[1]].tensor_copy(hi[:], hi8[:])
                mms(k, lo[:], hi[:], False)

            order = _completion_order()
            for k in order:
                unpack_and_mm(k)
            for pi in range(3):
                for j in range(3 if pi == 0 else 2):
                    fix_piece(j, pi)

            o1 = out[: 6 * 128].rearrange("(t p) b -> p t b", p=128)
            nc.sync.dma_start(o1, yt[:, : 6 * B].rearrange("p (t b) -> p t b", b=B))
            o2 = out[6 * 128:].rearrange("(t p) b -> p t b", p=128)
            nc.sync.dma_start(o2, yt[:, 6 * B:].rearrange("p (t b) -> p t b", b=B))

    nc.compile()
    return nc


def _get_nc():
    global _nc_cache
    if _nc_cache is None:
        _nc_cache = _build_nc()
    return _nc_cache


def _prep_inputs(x, qweight, scales, zeros, bias):
    x = np.asarray(x)
    qweight = np.asarray(qweight)
    scales = np.asarray(scales)
    zeros = np.asarray(zeros)
    bias = np.asarray(bias)

    qb = qweight.astype(np.uint8)
    nib = np.empty((O, I), np.uint8)
    nib[:, 0::2] = qb & 15
    nib[:, 1::2] = qb >> 4
    nibT = np.ascontiguousarray(nib.T)                    # (4096, 11008)
    wq_packed = nibT[: I // 2] | (nibT[I // 2:] << 4)     # (2048, 11008)

    xt_host = np.ascontiguousarray(
        x.T.reshape(NG, 128, B).transpose(1, 0, 2).reshape(128, NG * B)
    ).astype(np.float16)
    xh8 = xt_host.astype(ml_dtypes.float8_e4m3)
    resid = (xt_host.astype(np.float32) - xh8.astype(np.float32)).astype(np.float16)
    xl8 = resid.astype(ml_dtypes.float8_e4m3)
    x8_host = np.concatenate([xh8, xl8], axis=1)

    Xg = x.astype(np.float64).T.reshape(NG, 128, B).sum(axis=1)   # (32, 8)

    packed_ks = [k for k in range(NJ) if ROUTES[k] != "f8"]
    f8_ks = [k for k in range(NJ) if ROUTES[k] == "f8"]
    pk_chunks = _pairs(packed_ks)
    f8_chunks = _pairs(f8_ks)

    in_maps = []
    for c in range(NCORES):
        sl = slice(c * OSH, (c + 1) * OSH)
        s_pad = np.zeros((OPAD, NG), np.float32)
        s_pad[:OSH] = scales[sl].astype(np.float32)
        z_pad = np.zeros((OPAD, NG), np.float32)
        z_pad[:OSH] = zeros[sl].astype(np.float32)
        b_pad = np.zeros((OPAD,), np.float64)
        b_pad[:OSH] = bias[sl].astype(np.float64)
        wq_c = np.ascontiguousarray(wq_packed[:, sl])

        im = {}
        for k0, n in pk_chunks:
            im[f"wq{k0}"] = np.ascontiguousarray(
                wq_c.reshape(NJ, 128, OPW)[k0:k0 + n])
        for k0, n in f8_chunks:
            parts = []
            for k in range(k0, k0 + n):
                blk = wq_c[k * 128:(k + 1) * 128]
                parts.append((blk & 15).astype(ml_dtypes.float8_e4m3))
                parts.append((blk >> 4).astype(ml_dtypes.float8_e4m3))
            im[f"f8_{k0}"] = np.ascontiguousarray(np.concatenate(parts, axis=1))

        sfix = np.zeros((OPAD, NG), np.float32)
        for k in range(NJ):
            clo, chi = colmap(k)
            sfix[:, clo] = s_pad[:, k]
            hsc = s_pad[:, k + 16]
            sfix[:, chi] = hsc if ROUTES[k] == "f8" else hsc / 16.0
        sfix_t = np.ascontiguousarray(
            sfix.reshape(OT, 128, NG).transpose(1, 0, 2).reshape(128, OT * NG)
        ).astype(np.float16)

        corr = (s_pad * z_pad).astype(np.float64) @ Xg    # (OPAD, B)
        corrb = (b_pad[:, None] - corr)
        corrb_t = np.ascontiguousarray(
            corrb.reshape(OT, 128, B).transpose(1, 0, 2).reshape(128, OT * B)
        ).astype(np.float32)

        cblob = np.concatenate([
            xt_host.view(np.uint8),
            x8_host.view(np.uint8),
            sfix_t.view(np.uint8),
            corrb_t.view(np.uint8),
        ], axis=1)
        assert cblob.shape == (128, 2080), cblob.shape
        im["cblob"] = np.ascontiguousarray(cblob)
        in_maps.append(im)
    return in_maps


def _gather(results):
    y = np.concatenate([r["out"][:OSH] for r in results], axis=0)
    return np.ascontiguousarray(y.T)


def kernel(x, qweight, scales, zeros, bias, _trace=False):
    nc = _get_nc()
    in_maps = _prep_inputs(x, qweight, scales, zeros, bias)
    res = run_bass_kernel_spmd(
        nc, in_maps, core_ids=list(range(NCORES)), trace=_trace
    )
    out = _gather(res.results)
    if _trace:
        return out, res
    return out



# revision 2
# speedup vs baseline: 1.2640x; 1.2640x over previous
"""GPTQ 4-bit quantized linear (column-parallel over 8 NeuronCores), v4.

y = x @ dequant(qweight, scales, zeros).T + bias with byte-packed 4-bit pairs.

Core trick: fp8e4m3 bit patterns 0..15 decode to exactly b * 2^-9 (denormals
plus the first normal binade are uniformly spaced), so nibbles extracted with
cheap fused DVE bitwise ops (and / shr+and on u16 lanes) can be BITCAST to
fp8e4 and fed to the PE against an fp16 x (mixed-dtype matmul): no cast
instructions and no fp8 x-split. The 2^9 is folded into the host scales.

Layout per core (out_features 11008 -> 8 x 1376):
  * k-tiles 0-11 ship packed (0.5 B/weight = the DMA floor); k12-k15 ship as
    host-unpacked raw nibble bytes (same fp8-bitcast encoding, 2x bytes) so
    the tail of the stream needs no decode work. Stream order: wq0, cblob
    (x/scales/corr consts -- second so matmuls unblock early), wq2..wq10,
    then the f8 tiles, the last one alone so the final dependency is small.
  * D[o, t, c, b] per-group dots accumulate in PSUM; fix pieces [8,4,2,2]
    k-tiles: ACT copies PSUM to SBUF f16 [p,t,b,c] (transposing copy), the
    multiply by sfix runs in DVE 2x mode (b-broadcast, c innermost), a
    halving TT-tree reduces over c, and two alternating f16 accumulators
    (uA/uB) keep the serial chain short. Piece 2 multiplies/reduces on the
    otherwise-idle Pool engine (from SBUF; GPSIMD cannot touch PSUM), and
    the last piece multiplies straight from PSUM on DVE to skip the ACT hop.
  * Output: yt [128, 88] f16, one straight-copy DMA; host de-tiles.
"""

import numpy as np

import concourse.bacc as bacc
import concourse.mybir as mybir
import concourse.tile as tile
from concourse.bass_utils import run_bass_kernel_spmd

dt = mybir.dt
Alu = mybir.AluOpType

B = 8
I = 4096
O = 11008
NCORES = 8
OSH = O // NCORES            # 1376
OT = 11
OPW = OSH                    # 1376
NG = 32
NJ = 16                      # k-tiles (2048 / 128)

NPK = 12                     # packed k-tiles; k12-k15 ship as f8 bytes
PIECES = [8, 4, 2, 2]        # k-tiles per fix piece (2 groups per tile)
# hi-mask engine per packed pair: "v" = DVE, "p" = Pool (gpsimd)
HI_ROUTES = ["v", "v", "v", "v", "v", "v"]

_piece_of = []
_slot_of = []
for _pi, _n in enumerate(PIECES):
    for _j in range(_n):
        _piece_of.append(_pi)
        _slot_of.append(2 * _j)

_nc_cache = None


def _build_nc():
    nc = bacc.Bacc("TRN2", target_bir_lowering=False)

    wq_d = {k0: nc.dram_tensor(f"wq{k0}", [2, 128, OPW], dt.uint8,
                               kind="ExternalInput") for k0 in range(0, NPK, 2)}
    f8_d = {"f8t12": nc.dram_tensor("f8t12", [128, 4 * OPW], dt.uint8,
                                    kind="ExternalInput"),
            "f8t14": nc.dram_tensor("f8t14", [128, 2 * OPW], dt.uint8,
                                    kind="ExternalInput"),
            "f8t15": nc.dram_tensor("f8t15", [128, 2 * OPW], dt.uint8,
                                    kind="ExternalInput")}  # k14/k15 split
    cblob_d = nc.dram_tensor("cblob", [128, 1568], dt.uint8, kind="ExternalInput")
    out = nc.dram_tensor("out", [128, OT * B], dt.float16, kind="ExternalOutput")

    with tile.TileContext(nc) as tc:
        with (
            tc.tile_pool(name="const", bufs=1) as constp,
            tc.tile_pool(name="wqp", bufs=4) as wqp,
            tc.tile_pool(name="nibp", bufs=6) as nibp,
            tc.tile_pool(name="fixp", bufs=1) as fixp,
            tc.tile_pool(name="dpsp", bufs=1, space="PSUM") as dpsp,
        ):
            cblob = constp.tile([128, 1568], dt.uint8)
            wq_sb = {}

            def wq_dma(k0):
                wq_sb[k0] = wqp.tile([128, 2, OPW], dt.uint8, name=f"wq{k0}")
                nc.sync.dma_start(wq_sb[k0][:],
                                  wq_d[k0][:].rearrange("n p o -> p n o"))

            # stream: wq0, cblob, wq2..wq10, f8 tiles last
            wq_dma(0)
            nc.sync.dma_start(cblob[:], cblob_d[:])
            xt_sb = cblob[:, 0:512].bitcast(dt.float16)          # [128, 32*8]
            sfix_sb = cblob[:, 512:1216].bitcast(dt.float16)     # [128, 11*32]
            corrb_sb = cblob[:, 1216:1392].bitcast(dt.float16)   # [128, 11*8]
            for k0 in (2, 4, 6, 8, 10):
                wq_dma(k0)
            f8_sb = {}
            f8_sb["f8t12"] = constp.tile([128, 4 * OPW], dt.uint8, name="f8t12")
            nc.sync.dma_start(f8_sb["f8t12"][:], f8_d["f8t12"][:])
            for nm in ("f8t14", "f8t15"):
                f8_sb[nm] = constp.tile([128, 2 * OPW], dt.uint8, name=nm)
                nc.sync.dma_start(f8_sb[nm][:], f8_d[nm][:])

            # PSUM: piece tiles (3+2+1+1+1 banks)
            d_ps = [dpsp.tile([128, OT, 2 * n, 8], dt.float32, name=f"d{i}",
                              tag=f"d{i}") for i, n in enumerate(PIECES)]

            def decode_pair(k0):
                src16 = wq_sb[k0][:].rearrange("p n o -> p (n o)").bitcast(dt.uint16)
                ehi = nc.gpsimd if HI_ROUTES[k0 // 2] == "p" else nc.vector
                lo8 = nibp.tile([128, 2, OPW], dt.uint8, tag="lo8", name=f"lo8_{k0}")
                nc.vector.tensor_scalar(
                    lo8[:].rearrange("p n o -> p (n o)").bitcast(dt.uint16),
                    src16, 0x0F0F, None, op0=Alu.bitwise_and)
                hi8 = nibp.tile([128, 2, OPW], dt.uint8, tag="hi8", name=f"hi8_{k0}")
                ehi.tensor_scalar(
                    hi8[:].rearrange("p n o -> p (n o)").bitcast(dt.uint16),
                    src16, 4, 0x0F0F,
                    op0=Alu.logical_shift_right, op1=Alu.bitwise_and)
                return lo8, hi8

            def mms(k, lo_ap, hi_ap):
                # lo_ap/hi_ap: [128, OPW] uint8 views holding nibble values
                dtile = d_ps[_piece_of[k]]
                clo = _slot_of[k]
                glo, ghi = k, k + 16
                for t in range(OT):
                    w = 128 if t < OT - 1 else OPW - 128 * (OT - 1)
                    dv = dtile[: w, t]
                    nc.tensor.matmul(dv[:, clo],
                                     lo_ap[:, t * 128:t * 128 + w].bitcast(dt.float8e4),
                                     xt_sb[:, glo * B:(glo + 1) * B],
                                     start=True, stop=True)
                    nc.tensor.matmul(dv[:, clo + 1],
                                     hi_ap[:, t * 128:t * 128 + w].bitcast(dt.float8e4),
                                     xt_sb[:, ghi * B:(ghi + 1) * B],
                                     start=True, stop=True)

            uA = fixp.tile([128, OT * B], dt.float16, name="uA")
            uB = fixp.tile([128, OT * B], dt.float16, name="uB")
            yt = fixp.tile([128, OT * B], dt.float16, name="yt")

            def fix_piece(pi):
                veng = nc.gpsimd if pi == 2 else nc.vector
                ncols = 2 * PIECES[pi]
                c0 = sum(2 * n for n in PIECES[:pi])
                dv = d_ps[pi][:]
                sf = sfix_sb.rearrange("p (t c) -> p t c", c=NG)[
                    :, :, c0:c0 + ncols]
                tmp = fixp.tile([128, OT, B, ncols], dt.float16,
                                tag=f"tmp{pi}", name=f"tmp{pi}")
                if pi == 3:
                    # last piece: multiply straight from PSUM on DVE (1x but
                    # small; skips the ACT hop on the critical tail). GPSIMD
                    # cannot read PSUM, so only DVE pieces may do this.
                    nc.vector.tensor_tensor(
                        tmp[:], dv.transpose([0, 1, 3, 2]),
                        sf.unsqueeze(2).broadcast_to([128, OT, B, ncols]),
                        Alu.mult)
                else:
                    # ACT: PSUM f32 [p,t,c,b] -> SBUF f16 [p,t,b,c]
                    cp = fixp.tile([128, OT, B, ncols], dt.float16,
                                   tag=f"cp{pi}", name=f"cp{pi}")
                    nc.scalar.copy(cp[:].transpose([0, 1, 3, 2]), dv)
                    veng.tensor_tensor(
                        tmp[:], cp[:],
                        sf.unsqueeze(2).broadcast_to([128, OT, B, ncols]),
                        Alu.mult)
                # halve over c down to 2 columns (2x TT tree), then fold the
                # piece into one of two alternating accumulator chains
                with nc.allow_low_precision(reason="f16 partial sums"):
                    w = ncols
                    while w > 2:
                        h = w // 2
                        veng.tensor_tensor(
                            tmp[:, :, :, 0:h], tmp[:, :, :, 0:h],
                            tmp[:, :, :, h:w], Alu.add)
                        w = h
                    red = fixp.tile([128, OT, B], dt.float16, tag=f"red{pi}",
                                    name=f"red{pi}")
                    veng.tensor_tensor(red[:], tmp[:, :, :, 0],
                                       tmp[:, :, :, 1], Alu.add)
                    rv = red[:].rearrange("p t b -> p (t b)")
                    npc = len(PIECES)
                    if pi == 0:
                        nc.vector.tensor_tensor(uA[:], corrb_sb, rv, Alu.add)
                    elif pi == 1:
                        nc.vector.tensor_copy(uB[:], rv)
                    elif pi == npc - 1:
                        # last piece: yt = uAB + red (uAB folded in at npc-2)
                        nc.vector.tensor_tensor(yt[:], uA[:], rv, Alu.add)
                    elif pi == npc - 2:
                        veng.tensor_tensor(uB[:], uB[:], rv, Alu.add)
                        nc.vector.tensor_tensor(uA[:], uA[:], uB[:], Alu.add)
                    else:
                        eng = uA if pi % 2 == 0 else uB
                        nc.vector.tensor_tensor(eng[:], eng[:], rv, Alu.add)

            npieces = len(PIECES)
            fix_after = {}
            acc = 0
            for pi, n in enumerate(PIECES):
                acc += n
                fix_after[acc - 1] = pi

            for k0 in range(0, NPK, 2):
                lo8, hi8 = decode_pair(k0)
                for j in (0, 1):
                    k = k0 + j
                    mms(k, lo8[:, j], hi8[:, j])
                    if k in fix_after:
                        fix_piece(fix_after[k])
            # f8 tiles k12-k15 (nibble bytes: [lo hi] per tile)
            for j, k in enumerate((12, 13)):
                mms(k, f8_sb["f8t12"][:, (2 * j) * OPW:(2 * j + 1) * OPW],
                    f8_sb["f8t12"][:, (2 * j + 1) * OPW:(2 * j + 2) * OPW])
                if k in fix_after:
                    fix_piece(fix_after[k])
            for k in (14, 15):
                t = f8_sb[f"f8t{k}"]
                mms(k, t[:, 0:OPW], t[:, OPW:2 * OPW])
                if k in fix_after:
                    fix_piece(fix_after[k])

            nc.sync.dma_start(out[:], yt[:])

    nc.compile()
    return nc


def _get_nc():
    global _nc_cache
    if _nc_cache is None:
        _nc_cache = _build_nc()
    return _nc_cache


def _prep_inputs(x, qweight, scales, zeros, bias):
    x = np.asarray(x)
    qweight = np.asarray(qweight)
    scales = np.asarray(scales)
    zeros = np.asarray(zeros)
    bias = np.asarray(bias)

    qb = qweight.astype(np.uint8)
    nib = np.empty((O, I), np.uint8)
    nib[:, 0::2] = qb & 15
    nib[:, 1::2] = qb >> 4
    nibT = np.ascontiguousarray(nib.T)                    # (4096, 11008)
    wq_packed = nibT[: I // 2] | (nibT[I // 2:] << 4)     # (2048, 11008)

    xt_host = np.ascontiguousarray(
        x.T.reshape(NG, 128, B).transpose(1, 0, 2).reshape(128, NG * B)
    ).astype(np.float16)

    Xg = x.astype(np.float64).T.reshape(NG, 128, B).sum(axis=1)   # (32, 8)

    piece_base = []
    acc = 0
    for n in PIECES:
        piece_base.append(acc)
        acc += 2 * n
    col_lo = {}
    for k in range(NJ):
        col_lo[k] = piece_base[_piece_of[k]] + _slot_of[k]

    in_maps = []
    for cidx in range(NCORES):
        sl = slice(cidx * OSH, (cidx + 1) * OSH)
        s_c = scales[sl].astype(np.float64)      # (1376, 32)
        z_c = zeros[sl].astype(np.float64)
        b_c = bias[sl].astype(np.float64)
        wq_c = np.ascontiguousarray(wq_packed[:, sl])

        im = {}
        kt = wq_c.reshape(NJ, 128, OPW)
        for k0 in range(0, NPK, 2):
            im[f"wq{k0}"] = np.ascontiguousarray(kt[k0:k0 + 2])
        im["f8t12"] = np.ascontiguousarray(np.concatenate(
            [kt[12] & 15, kt[12] >> 4, kt[13] & 15, kt[13] >> 4], axis=1))
        for k in (14, 15):
            im[f"f8t{k}"] = np.ascontiguousarray(np.concatenate(
                [kt[k] & 15, kt[k] >> 4], axis=1))

        # sfix: value = s * 512 (fp8 bitcast carries 2^-9)
        sfix = np.zeros((OT * 128, NG), np.float64)
        for k in range(NJ):
            sfix[:OSH, col_lo[k]] = s_c[:, k] * 512.0
            sfix[:OSH, col_lo[k] + 1] = s_c[:, k + 16] * 512.0
        sfix_t = np.ascontiguousarray(
            sfix.reshape(OT, 128, NG).transpose(1, 0, 2).reshape(128, OT * NG)
        ).astype(np.float16)

        corr = (s_c * z_c) @ Xg                  # (1376, 8)
        corrb = np.zeros((OT * 128, B), np.float64)
        corrb[:OSH] = b_c[:, None] - corr
        corrb_t = np.ascontiguousarray(
            corrb.reshape(OT, 128, B).transpose(1, 0, 2).reshape(128, OT * B)
        ).astype(np.float16)

        cblob = np.concatenate([
            xt_host.view(np.uint8),
            sfix_t.view(np.uint8),
            corrb_t.view(np.uint8),
            np.zeros((128, 176), np.uint8),
        ], axis=1)
        assert cblob.shape == (128, 1568), cblob.shape
        im["cblob"] = np.ascontiguousarray(cblob)
        in_maps.append(im)
    return in_maps


def _gather(results):
    outs = []
    for r in results:
        o = r["out"]                              # (128, 88) f16
        y = o.reshape(128, OT, B).transpose(1, 0, 2).reshape(OT * 128, B)
        outs.append(y[:OSH])
    y = np.concatenate(outs, axis=0)
    return np.ascontiguousarray(y.T)


def kernel(x, qweight, scales, zeros, bias, _trace=False):
    nc = _get_nc()
    in_maps = _prep_inputs(x, qweight, scales, zeros, bias)
    res = run_bass_kernel_spmd(
        nc, in_maps, core_ids=list(range(NCORES)), trace=_trace
    )
    out = _gather(res.results)
    if _trace:
        return out, res
    return out


# revision 3
# speedup vs baseline: 1.2714x; 1.0059x over previous
"""GPTQ 4-bit quantized linear (column-parallel over 8 NeuronCores), v4.

y = x @ dequant(qweight, scales, zeros).T + bias with byte-packed 4-bit pairs.

Core trick: fp8e4m3 bit patterns 0..15 decode to exactly b * 2^-9 (denormals
plus the first normal binade are uniformly spaced), so nibbles extracted with
cheap fused DVE bitwise ops (and / shr+and on u16 lanes) can be BITCAST to
fp8e4 and fed to the PE against an fp16 x (mixed-dtype matmul): no cast
instructions and no fp8 x-split. The 2^9 is folded into the host scales.

Layout per core (out_features 11008 -> 8 x 1376):
  * k-tiles 0-13 ship packed (0.5 B/weight); the last pair (k14,15) ships as
    host-unpacked raw nibble bytes (same fp8-bitcast encoding, 2x bytes) so
    the tail of the stream needs no decode work at all.
  * Pool (gpsimd) absorbs the hi-nibble masks of a few early pairs.
  * D[o, t, c, b] per-group dots accumulate in PSUM; fix pieces [8,2,2,2,2]
    k-tiles: ACT copies PSUM to SBUF f16 [p,t,b,c] (transposing copy), DVE
    multiplies by sfix in 2x mode (b-broadcast, c innermost), halving TT-tree
    reduces over c, two alternating f16 accumulators. Piece 2 runs on the
    otherwise-idle Pool engine (small enough not to gate the tail).
  * Output: yt [128, 88] f16, one straight-copy DMA; host de-tiles.
"""

import numpy as np

import concourse.bacc as bacc
import concourse.mybir as mybir
import concourse.tile as tile
from concourse.bass_utils import run_bass_kernel_spmd

dt = mybir.dt
Alu = mybir.AluOpType

B = 8
I = 4096
O = 11008
NCORES = 8
OSH = O // NCORES            # 1376
OT = 11
OPW = OSH                    # 1376
NG = 32
NJ = 16                      # k-tiles (2048 / 128)

NPK = 12                     # packed k-tiles; k12-k15 ship as f8 bytes
PIECES = [8, 2, 2, 2, 2]     # k-tiles per fix piece (2 groups per tile)
# hi-mask engine per packed pair: "v" = DVE, "p" = Pool (gpsimd)
HI_ROUTES = ["v", "v", "v", "v", "v", "v"]

_piece_of = []
_slot_of = []
for _pi, _n in enumerate(PIECES):
    for _j in range(_n):
        _piece_of.append(_pi)
        _slot_of.append(2 * _j)

_nc_cache = None


def _build_nc():
    nc = bacc.Bacc("TRN2", target_bir_lowering=False)

    wq_d = {k0: nc.dram_tensor(f"wq{k0}", [2, 128, OPW], dt.uint8,
                               kind="ExternalInput") for k0 in range(0, NPK, 2)}
    f8_d = {"f8t12": nc.dram_tensor("f8t12", [128, 4 * OPW], dt.uint8,
                                    kind="ExternalInput"),
            "f8t14": nc.dram_tensor("f8t14", [128, 2 * OPW], dt.uint8,
                                    kind="ExternalInput"),
            "f8t15": nc.dram_tensor("f8t15", [128, 2 * OPW], dt.uint8,
                                    kind="ExternalInput")}  # k14/k15 split
    cblob_d = nc.dram_tensor("cblob", [128, 1568], dt.uint8, kind="ExternalInput")
    out = nc.dram_tensor("out", [128, OT * B], dt.float16, kind="ExternalOutput")

    with tile.TileContext(nc) as tc:
        with (
            tc.tile_pool(name="const", bufs=1) as constp,
            tc.tile_pool(name="wqp", bufs=4) as wqp,
            tc.tile_pool(name="nibp", bufs=6) as nibp,
            tc.tile_pool(name="fixp", bufs=1) as fixp,
            tc.tile_pool(name="dpsp", bufs=1, space="PSUM") as dpsp,
        ):
            cblob = constp.tile([128, 1568], dt.uint8)
            wq_sb = {}

            def wq_dma(k0):
                wq_sb[k0] = wqp.tile([128, 2, OPW], dt.uint8, name=f"wq{k0}")
                nc.sync.dma_start(wq_sb[k0][:],
                                  wq_d[k0][:].rearrange("n p o -> p n o"))

            # stream: wq0, cblob, wq2..wq10, f8 tiles last
            wq_dma(0)
            nc.sync.dma_start(cblob[:], cblob_d[:])
            xt_sb = cblob[:, 0:512].bitcast(dt.float16)          # [128, 32*8]
            sfix_sb = cblob[:, 512:1216].bitcast(dt.float16)     # [128, 11*32]
            corrb_sb = cblob[:, 1216:1392].bitcast(dt.float16)   # [128, 11*8]
            for k0 in (2, 4, 6, 8, 10):
                wq_dma(k0)
            f8_sb = {}
            f8_sb["f8t12"] = constp.tile([128, 4 * OPW], dt.uint8, name="f8t12")
            nc.sync.dma_start(f8_sb["f8t12"][:], f8_d["f8t12"][:])
            for nm in ("f8t14", "f8t15"):
                f8_sb[nm] = constp.tile([128, 2 * OPW], dt.uint8, name=nm)
                nc.sync.dma_start(f8_sb[nm][:], f8_d[nm][:])

            # PSUM: piece tiles (3+2+1+1+1 banks)
            d_ps = [dpsp.tile([128, OT, 2 * n, 8], dt.float32, name=f"d{i}",
                              tag=f"d{i}") for i, n in enumerate(PIECES)]

            def decode_pair(k0):
                src16 = wq_sb[k0][:].rearrange("p n o -> p (n o)").bitcast(dt.uint16)
                ehi = nc.gpsimd if HI_ROUTES[k0 // 2] == "p" else nc.vector
                lo8 = nibp.tile([128, 2, OPW], dt.uint8, tag="lo8", name=f"lo8_{k0}")
                nc.vector.tensor_scalar(
                    lo8[:].rearrange("p n o -> p (n o)").bitcast(dt.uint16),
                    src16, 0x0F0F, None, op0=Alu.bitwise_and)
                hi8 = nibp.tile([128, 2, OPW], dt.uint8, tag="hi8", name=f"hi8_{k0}")
                ehi.tensor_scalar(
                    hi8[:].rearrange("p n o -> p (n o)").bitcast(dt.uint16),
                    src16, 4, 0x0F0F,
                    op0=Alu.logical_shift_right, op1=Alu.bitwise_and)
                return lo8, hi8

            def mms(k, lo_ap, hi_ap):
                # lo_ap/hi_ap: [128, OPW] uint8 views holding nibble values
                dtile = d_ps[_piece_of[k]]
                clo = _slot_of[k]
                glo, ghi = k, k + 16
                for t in range(OT):
                    w = 128 if t < OT - 1 else OPW - 128 * (OT - 1)
                    dv = dtile[: w, t]
                    nc.tensor.matmul(dv[:, clo],
                                     lo_ap[:, t * 128:t * 128 + w].bitcast(dt.float8e4),
                                     xt_sb[:, glo * B:(glo + 1) * B],
                                     start=True, stop=True)
                    nc.tensor.matmul(dv[:, clo + 1],
                                     hi_ap[:, t * 128:t * 128 + w].bitcast(dt.float8e4),
                                     xt_sb[:, ghi * B:(ghi + 1) * B],
                                     start=True, stop=True)

            uA = fixp.tile([128, OT * B], dt.float16, name="uA")
            uB = fixp.tile([128, OT * B], dt.float16, name="uB")
            yt = fixp.tile([128, OT * B], dt.float16, name="yt")

            def fix_piece(pi):
                veng = nc.gpsimd if pi == 2 else nc.vector
                ncols = 2 * PIECES[pi]
                c0 = sum(2 * n for n in PIECES[:pi])
                dv = d_ps[pi][:]
                sf = sfix_sb.rearrange("p (t c) -> p t c", c=NG)[
                    :, :, c0:c0 + ncols]
                tmp = fixp.tile([128, OT, B, ncols], dt.float16,
                                tag=f"tmp{pi}", name=f"tmp{pi}")
                if pi == len(PIECES) - 1:
                    # last piece: multiply straight from PSUM on DVE (1x but
                    # small; skips the ACT hop on the critical tail). GPSIMD
                    # cannot read PSUM, so only DVE pieces may do this.
                    nc.vector.tensor_tensor(
                        tmp[:], dv.transpose([0, 1, 3, 2]),
                        sf.unsqueeze(2).broadcast_to([128, OT, B, ncols]),
                        Alu.mult)
                else:
                    # ACT: PSUM f32 [p,t,c,b] -> SBUF f16 [p,t,b,c]
                    cp = fixp.tile([128, OT, B, ncols], dt.float16,
                                   tag=f"cp{pi}", name=f"cp{pi}")
                    nc.scalar.copy(cp[:].transpose([0, 1, 3, 2]), dv)
                    veng.tensor_tensor(
                        tmp[:], cp[:],
                        sf.unsqueeze(2).broadcast_to([128, OT, B, ncols]),
                        Alu.mult)
                # halve over c down to 2 columns (2x TT tree), then fold the
                # piece into one of two alternating accumulator chains
                with nc.allow_low_precision(reason="f16 partial sums"):
                    w = ncols
                    while w > 2:
                        h = w // 2
                        veng.tensor_tensor(
                            tmp[:, :, :, 0:h], tmp[:, :, :, 0:h],
                            tmp[:, :, :, h:w], Alu.add)
                        w = h
                    red = fixp.tile([128, OT, B], dt.float16, tag=f"red{pi}",
                                    name=f"red{pi}")
                    veng.tensor_tensor(red[:], tmp[:, :, :, 0],
                                       tmp[:, :, :, 1], Alu.add)
                    rv = red[:].rearrange("p t b -> p (t b)")
                    npc = len(PIECES)
                    if pi == 0:
                        nc.vector.tensor_tensor(uA[:], corrb_sb, rv, Alu.add)
                    elif pi == 1:
                        nc.vector.tensor_copy(uB[:], rv)
                    elif pi == npc - 1:
                        # last piece: yt = uAB + red (uAB folded in at npc-2)
                        nc.vector.tensor_tensor(yt[:], uA[:], rv, Alu.add)
                    elif pi == npc - 2:
                        veng.tensor_tensor(uB[:], uB[:], rv, Alu.add)
                        nc.vector.tensor_tensor(uA[:], uA[:], uB[:], Alu.add)
                    else:
                        eng = uA if pi % 2 == 0 else uB
                        nc.vector.tensor_tensor(eng[:], eng[:], rv, Alu.add)

            npieces = len(PIECES)
            fix_after = {}
            acc = 0
            for pi, n in enumerate(PIECES):
                acc += n
                fix_after[acc - 1] = pi

            for k0 in range(0, NPK, 2):
                lo8, hi8 = decode_pair(k0)
                for j in (0, 1):
                    k = k0 + j
                    mms(k, lo8[:, j], hi8[:, j])
                    if k in fix_after:
                        fix_piece(fix_after[k])
            # f8 tiles k12-k15 (nibble bytes: [lo hi] per tile)
            for j, k in enumerate((12, 13)):
                mms(k, f8_sb["f8t12"][:, (2 * j) * OPW:(2 * j + 1) * OPW],
                    f8_sb["f8t12"][:, (2 * j + 1) * OPW:(2 * j + 2) * OPW])
                if k in fix_after:
                    fix_piece(fix_after[k])
            for k in (14, 15):
                t = f8_sb[f"f8t{k}"]
                mms(k, t[:, 0:OPW], t[:, OPW:2 * OPW])
                if k in fix_after:
                    fix_piece(fix_after[k])

            nc.sync.dma_start(out[:], yt[:])

    nc.compile()
    return nc


def _get_nc():
    global _nc_cache
    if _nc_cache is None:
        _nc_cache = _build_nc()
    return _nc_cache


def _prep_inputs(x, qweight, scales, zeros, bias):
    x = np.asarray(x)
    qweight = np.asarray(qweight)
    scales = np.asarray(scales)
    zeros = np.asarray(zeros)
    bias = np.asarray(bias)

    qb = qweight.astype(np.uint8)
    nib = np.empty((O, I), np.uint8)
    nib[:, 0::2] = qb & 15
    nib[:, 1::2] = qb >> 4
    nibT = np.ascontiguousarray(nib.T)                    # (4096, 11008)
    wq_packed = nibT[: I // 2] | (nibT[I // 2:] << 4)     # (2048, 11008)

    xt_host = np.ascontiguousarray(
        x.T.reshape(NG, 128, B).transpose(1, 0, 2).reshape(128, NG * B)
    ).astype(np.float16)

    Xg = x.astype(np.float64).T.reshape(NG, 128, B).sum(axis=1)   # (32, 8)

    piece_base = []
    acc = 0
    for n in PIECES:
        piece_base.append(acc)
        acc += 2 * n
    col_lo = {}
    for k in range(NJ):
        col_lo[k] = piece_base[_piece_of[k]] + _slot_of[k]

    in_maps = []
    for cidx in range(NCORES):
        sl = slice(cidx * OSH, (cidx + 1) * OSH)
        s_c = scales[sl].astype(np.float64)      # (1376, 32)
        z_c = zeros[sl].astype(np.float64)
        b_c = bias[sl].astype(np.float64)
        wq_c = np.ascontiguousarray(wq_packed[:, sl])

        im = {}
        kt = wq_c.reshape(NJ, 128, OPW)
        for k0 in range(0, NPK, 2):
            im[f"wq{k0}"] = np.ascontiguousarray(kt[k0:k0 + 2])
        im["f8t12"] = np.ascontiguousarray(np.concatenate(
            [kt[12] & 15, kt[12] >> 4, kt[13] & 15, kt[13] >> 4], axis=1))
        for k in (14, 15):
            im[f"f8t{k}"] = np.ascontiguousarray(np.concatenate(
                [kt[k] & 15, kt[k] >> 4], axis=1))

        # sfix: value = s * 512 (fp8 bitcast carries 2^-9)
        sfix = np.zeros((OT * 128, NG), np.float64)
        for k in range(NJ):
            sfix[:OSH, col_lo[k]] = s_c[:, k] * 512.0
            sfix[:OSH, col_lo[k] + 1] = s_c[:, k + 16] * 512.0
        sfix_t = np.ascontiguousarray(
            sfix.reshape(OT, 128, NG).transpose(1, 0, 2).reshape(128, OT * NG)
        ).astype(np.float16)

        corr = (s_c * z_c) @ Xg                  # (1376, 8)
        corrb = np.zeros((OT * 128, B), np.float64)
        corrb[:OSH] = b_c[:, None] - corr
        corrb_t = np.ascontiguousarray(
            corrb.reshape(OT, 128, B).transpose(1, 0, 2).reshape(128, OT * B)
        ).astype(np.float16)

        cblob = np.concatenate([
            xt_host.view(np.uint8),
            sfix_t.view(np.uint8),
            corrb_t.view(np.uint8),
            np.zeros((128, 176), np.uint8),
        ], axis=1)
        assert cblob.shape == (128, 1568), cblob.shape
        im["cblob"] = np.ascontiguousarray(cblob)
        in_maps.append(im)
    return in_maps


def _gather(results):
    outs = []
    for r in results:
        o = r["out"]                              # (128, 88) f16
        y = o.reshape(128, OT, B).transpose(1, 0, 2).reshape(OT * 128, B)
        outs.append(y[:OSH])
    y = np.concatenate(outs, axis=0)
    return np.ascontiguousarray(y.T)


def kernel(x, qweight, scales, zeros, bias, _trace=False):
    nc = _get_nc()
    in_maps = _prep_inputs(x, qweight, scales, zeros, bias)
    res = run_bass_kernel_spmd(
        nc, in_maps, core_ids=list(range(NCORES)), trace=_trace
    )
    out = _gather(res.results)
    if _trace:
        return out, res
    return out


# revision 4
# speedup vs baseline: 1.2738x; 1.0018x over previous
"""GPTQ 4-bit quantized linear (column-parallel over 8 NeuronCores), v4.

y = x @ dequant(qweight, scales, zeros).T + bias with byte-packed 4-bit pairs.

Core trick: fp8e4m3 bit patterns 0..15 decode to exactly b * 2^-9 (denormals
plus the first normal binade are uniformly spaced), so nibbles extracted with
cheap fused DVE bitwise ops (and / shr+and on u16 lanes) can be BITCAST to
fp8e4 and fed to the PE against an fp16 x (mixed-dtype matmul): no cast
instructions and no fp8 x-split. The 2^9 is folded into the host scales.

Layout per core (out_features 11008 -> 8 x 1376):
  * k-tiles 0-13 ship packed (0.5 B/weight); the last pair (k14,15) ships as
    host-unpacked raw nibble bytes (same fp8-bitcast encoding, 2x bytes) so
    the tail of the stream needs no decode work at all.
  * Pool (gpsimd) absorbs the hi-nibble masks of a few early pairs.
  * D[o, t, c, b] per-group dots accumulate in PSUM; fix pieces [8,4,2,2]
    k-tiles: ACT copies PSUM to SBUF f16 [p,t,b,c] (transposing copy), DVE
    multiplies by sfix in 2x mode (b-broadcast, c innermost), halving TT-tree
    reduces over c, two f16 accumulator adds per piece.
  * Output: yt [128, 88] f16, one straight-copy DMA; host de-tiles.
"""

import numpy as np

import concourse.bacc as bacc
import concourse.mybir as mybir
import concourse.tile as tile
from concourse.bass_utils import run_bass_kernel_spmd

dt = mybir.dt
Alu = mybir.AluOpType

B = 8
I = 4096
O = 11008
NCORES = 8
OSH = O // NCORES            # 1376
OT = 11
OPW = OSH                    # 1376
NG = 32
NJ = 16                      # k-tiles (2048 / 128)

NPK = 12                     # packed k-tiles; k12-k15 ship as f8 bytes
PIECES = [8, 2, 2, 2, 2]     # k-tiles per fix piece (2 groups per tile)
# hi-mask engine per packed pair: "v" = DVE, "p" = Pool (gpsimd)
HI_ROUTES = ["v", "v", "v", "v", "v", "v"]

_piece_of = []
_slot_of = []
for _pi, _n in enumerate(PIECES):
    for _j in range(_n):
        _piece_of.append(_pi)
        _slot_of.append(2 * _j)

_nc_cache = None


def _build_nc():
    nc = bacc.Bacc("TRN2", target_bir_lowering=False)

    wq_d = {k0: nc.dram_tensor(f"wq{k0}", [2, 128, OPW], dt.uint8,
                               kind="ExternalInput") for k0 in range(0, NPK, 2)}
    f8_d = {"f8t12": nc.dram_tensor("f8t12", [128, 4 * OPW], dt.uint8,
                                    kind="ExternalInput"),
            "f8t14": nc.dram_tensor("f8t14", [128, 2 * OPW], dt.uint8,
                                    kind="ExternalInput"),
            "f8t15": nc.dram_tensor("f8t15", [128, 2 * OPW], dt.uint8,
                                    kind="ExternalInput")}  # k14/k15 split
    cblob_d = nc.dram_tensor("cblob", [128, 1568], dt.uint8, kind="ExternalInput")
    out = nc.dram_tensor("out", [128, OT * B], dt.float16, kind="ExternalOutput")

    with tile.TileContext(nc) as tc:
        with (
            tc.tile_pool(name="const", bufs=1) as constp,
            tc.tile_pool(name="wqp", bufs=4) as wqp,
            tc.tile_pool(name="nibp", bufs=6) as nibp,
            tc.tile_pool(name="fixp", bufs=1) as fixp,
            tc.tile_pool(name="dpsp", bufs=1, space="PSUM") as dpsp,
        ):
            cblob = constp.tile([128, 1568], dt.uint8)
            wq_sb = {}

            def wq_dma(k0):
                wq_sb[k0] = wqp.tile([128, 2, OPW], dt.uint8, name=f"wq{k0}")
                nc.sync.dma_start(wq_sb[k0][:],
                                  wq_d[k0][:].rearrange("n p o -> p n o"))

            # stream: wq0, cblob, wq2..wq10, f8 tiles last
            wq_dma(0)
            nc.sync.dma_start(cblob[:], cblob_d[:])
            xt_sb = cblob[:, 0:512].bitcast(dt.float16)          # [128, 32*8]
            sfix_sb = cblob[:, 512:1216].bitcast(dt.float16)     # [128, 11*32]
            corrb_sb = cblob[:, 1216:1392].bitcast(dt.float16)   # [128, 11*8]
            for k0 in (2, 4, 6, 8, 10):
                wq_dma(k0)
            f8_sb = {}
            f8_sb["f8t12"] = constp.tile([128, 4 * OPW], dt.uint8, name="f8t12")
            nc.sync.dma_start(f8_sb["f8t12"][:], f8_d["f8t12"][:])
            for nm in ("f8t14", "f8t15"):
                f8_sb[nm] = constp.tile([128, 2 * OPW], dt.uint8, name=nm)
                nc.sync.dma_start(f8_sb[nm][:], f8_d[nm][:])

            # PSUM: piece tiles (3+2+1+1+1 banks)
            d_ps = [dpsp.tile([128, OT, 2 * n, 8], dt.float32, name=f"d{i}",
                              tag=f"d{i}") for i, n in enumerate(PIECES)]

            def decode_pair(k0):
                src16 = wq_sb[k0][:].rearrange("p n o -> p (n o)").bitcast(dt.uint16)
                ehi = nc.gpsimd if HI_ROUTES[k0 // 2] == "p" else nc.vector
                lo8 = nibp.tile([128, 2, OPW], dt.uint8, tag="lo8", name=f"lo8_{k0}")
                nc.vector.tensor_scalar(
                    lo8[:].rearrange("p n o -> p (n o)").bitcast(dt.uint16),
                    src16, 0x0F0F, None, op0=Alu.bitwise_and)
                hi8 = nibp.tile([128, 2, OPW], dt.uint8, tag="hi8", name=f"hi8_{k0}")
                ehi.tensor_scalar(
                    hi8[:].rearrange("p n o -> p (n o)").bitcast(dt.uint16),
                    src16, 4, 0x0F0F,
                    op0=Alu.logical_shift_right, op1=Alu.bitwise_and)
                return lo8, hi8

            def mms(k, lo_ap, hi_ap):
                # lo_ap/hi_ap: [128, OPW] uint8 views holding nibble values
                dtile = d_ps[_piece_of[k]]
                clo = _slot_of[k]
                glo, ghi = k, k + 16
                for t in range(OT):
                    w = 128 if t < OT - 1 else OPW - 128 * (OT - 1)
                    dv = dtile[: w, t]
                    nc.tensor.matmul(dv[:, clo],
                                     lo_ap[:, t * 128:t * 128 + w].bitcast(dt.float8e4),
                                     xt_sb[:, glo * B:(glo + 1) * B],
                                     start=True, stop=True)
                    nc.tensor.matmul(dv[:, clo + 1],
                                     hi_ap[:, t * 128:t * 128 + w].bitcast(dt.float8e4),
                                     xt_sb[:, ghi * B:(ghi + 1) * B],
                                     start=True, stop=True)

            uA = fixp.tile([128, OT * B], dt.float16, name="uA")
            uB = fixp.tile([128, OT * B], dt.float16, name="uB")
            yt = fixp.tile([128, OT * B], dt.float16, name="yt")

            def fix_piece(pi):
                veng = nc.gpsimd if pi == 2 else nc.vector
                ncols = 2 * PIECES[pi]
                c0 = sum(2 * n for n in PIECES[:pi])
                dv = d_ps[pi][:]
                sf = sfix_sb.rearrange("p (t c) -> p t c", c=NG)[
                    :, :, c0:c0 + ncols]
                tmp = fixp.tile([128, OT, B, ncols], dt.float16,
                                tag=f"tmp{pi}", name=f"tmp{pi}")
                if pi == len(PIECES) - 1:
                    # last piece: multiply straight from PSUM on DVE (1x but
                    # small; skips the ACT hop on the critical tail). GPSIMD
                    # cannot read PSUM, so only DVE pieces may do this.
                    nc.vector.tensor_tensor(
                        tmp[:], dv.transpose([0, 1, 3, 2]),
                        sf.unsqueeze(2).broadcast_to([128, OT, B, ncols]),
                        Alu.mult)
                else:
                    # ACT: PSUM f32 [p,t,c,b] -> SBUF f16 [p,t,b,c]
                    cp = fixp.tile([128, OT, B, ncols], dt.float16,
                                   tag=f"cp{pi}", name=f"cp{pi}")
                    nc.scalar.copy(cp[:].transpose([0, 1, 3, 2]), dv)
                    veng.tensor_tensor(
                        tmp[:], cp[:],
                        sf.unsqueeze(2).broadcast_to([128, OT, B, ncols]),
                        Alu.mult)
                # halve over c down to 2 columns (2x TT tree), then fold the
                # piece into one of two alternating accumulator chains
                with nc.allow_low_precision(reason="f16 partial sums"):
                    w = ncols
                    while w > 2:
                        h = w // 2
                        veng.tensor_tensor(
                            tmp[:, :, :, 0:h], tmp[:, :, :, 0:h],
                            tmp[:, :, :, h:w], Alu.add)
                        w = h
                    red = fixp.tile([128, OT, B], dt.float16, tag=f"red{pi}",
                                    name=f"red{pi}")
                    veng.tensor_tensor(red[:], tmp[:, :, :, 0],
                                       tmp[:, :, :, 1], Alu.add)
                    rv = red[:].rearrange("p t b -> p (t b)")
                    npc = len(PIECES)
                    if pi == 0:
                        nc.vector.tensor_tensor(uA[:], corrb_sb, rv, Alu.add)
                    elif pi == 1:
                        nc.vector.tensor_copy(uB[:], rv)
                    elif pi == npc - 1:
                        # last piece: yt = uAB + red (uAB folded in at npc-2)
                        nc.vector.tensor_tensor(yt[:], uA[:], rv, Alu.add)
                    elif pi == npc - 2:
                        # fold uB on the idle Pool engine, off the DVE tail
                        nc.gpsimd.tensor_tensor(uB[:], uB[:], rv, Alu.add)
                        nc.vector.tensor_tensor(uA[:], uA[:], uB[:], Alu.add)
                    else:
                        eng = uA if pi % 2 == 0 else uB
                        nc.vector.tensor_tensor(eng[:], eng[:], rv, Alu.add)

            npieces = len(PIECES)
            fix_after = {}
            acc = 0
            for pi, n in enumerate(PIECES):
                acc += n
                fix_after[acc - 1] = pi

            for k0 in range(0, NPK, 2):
                lo8, hi8 = decode_pair(k0)
                for j in (0, 1):
                    k = k0 + j
                    mms(k, lo8[:, j], hi8[:, j])
                    if k in fix_after:
                        fix_piece(fix_after[k])
            # f8 tiles k12-k15 (nibble bytes: [lo hi] per tile)
            for j, k in enumerate((12, 13)):
                mms(k, f8_sb["f8t12"][:, (2 * j) * OPW:(2 * j + 1) * OPW],
                    f8_sb["f8t12"][:, (2 * j + 1) * OPW:(2 * j + 2) * OPW])
                if k in fix_after:
                    fix_piece(fix_after[k])
            for k in (14, 15):
                t = f8_sb[f"f8t{k}"]
                mms(k, t[:, 0:OPW], t[:, OPW:2 * OPW])
                if k in fix_after:
                    fix_piece(fix_after[k])

            nc.sync.dma_start(out[:], yt[:])

    nc.compile()
    return nc


def _get_nc():
    global _nc_cache
    if _nc_cache is None:
        _nc_cache = _build_nc()
    return _nc_cache


def _prep_inputs(x, qweight, scales, zeros, bias):
    x = np.asarray(x)
    qweight = np.asarray(qweight)
    scales = np.asarray(scales)
    zeros = np.asarray(zeros)
    bias = np.asarray(bias)

    qb = qweight.astype(np.uint8)
    nib = np.empty((O, I), np.uint8)
    nib[:, 0::2] = qb & 15
    nib[:, 1::2] = qb >> 4
    nibT = np.ascontiguousarray(nib.T)                    # (4096, 11008)
    wq_packed = nibT[: I // 2] | (nibT[I // 2:] << 4)     # (2048, 11008)

    xt_host = np.ascontiguousarray(
        x.T.reshape(NG, 128, B).transpose(1, 0, 2).reshape(128, NG * B)
    ).astype(np.float16)

    Xg = x.astype(np.float64).T.reshape(NG, 128, B).sum(axis=1)   # (32, 8)

    piece_base = []
    acc = 0
    for n in PIECES:
        piece_base.append(acc)
        acc += 2 * n
    col_lo = {}
    for k in range(NJ):
        col_lo[k] = piece_base[_piece_of[k]] + _slot_of[k]

    in_maps = []
    for cidx in range(NCORES):
        sl = slice(cidx * OSH, (cidx + 1) * OSH)
        s_c = scales[sl].astype(np.float64)      # (1376, 32)
        z_c = zeros[sl].astype(np.float64)
        b_c = bias[sl].astype(np.float64)
        wq_c = np.ascontiguousarray(wq_packed[:, sl])

        im = {}
        kt = wq_c.reshape(NJ, 128, OPW)
        for k0 in range(0, NPK, 2):
            im[f"wq{k0}"] = np.ascontiguousarray(kt[k0:k0 + 2])
        im["f8t12"] = np.ascontiguousarray(np.concatenate(
            [kt[12] & 15, kt[12] >> 4, kt[13] & 15, kt[13] >> 4], axis=1))
        for k in (14, 15):
            im[f"f8t{k}"] = np.ascontiguousarray(np.concatenate(
                [kt[k] & 15, kt[k] >> 4], axis=1))

        # sfix: value = s * 512 (fp8 bitcast carries 2^-9)
        sfix = np.zeros((OT * 128, NG), np.float64)
        for k in range(NJ):
            sfix[:OSH, col_lo[k]] = s_c[:, k] * 512.0
            sfix[:OSH, col_lo[k] + 1] = s_c[:, k + 16] * 512.0
        sfix_t = np.ascontiguousarray(
            sfix.reshape(OT, 128, NG).transpose(1, 0, 2).reshape(128, OT * NG)
        ).astype(np.float16)

        corr = (s_c * z_c) @ Xg                  # (1376, 8)
        corrb = np.zeros((OT * 128, B), np.float64)
        corrb[:OSH] = b_c[:, None] - corr
        corrb_t = np.ascontiguousarray(
            corrb.reshape(OT, 128, B).transpose(1, 0, 2).reshape(128, OT * B)
        ).astype(np.float16)

        cblob = np.concatenate([
            xt_host.view(np.uint8),
            sfix_t.view(np.uint8),
            corrb_t.view(np.uint8),
            np.zeros((128, 176), np.uint8),
        ], axis=1)
        assert cblob.shape == (128, 1568), cblob.shape
        im["cblob"] = np.ascontiguousarray(cblob)
        in_maps.append(im)
    return in_maps


def _gather(results):
    outs = []
    for r in results:
        o = r["out"]                              # (128, 88) f16
        y = o.reshape(128, OT, B).transpose(1, 0, 2).reshape(OT * 128, B)
        outs.append(y[:OSH])
    y = np.concatenate(outs, axis=0)
    return np.ascontiguousarray(y.T)


def kernel(x, qweight, scales, zeros, bias, _trace=False):
    nc = _get_nc()
    in_maps = _prep_inputs(x, qweight, scales, zeros, bias)
    res = run_bass_kernel_spmd(
        nc, in_maps, core_ids=list(range(NCORES)), trace=_trace
    )
    out = _gather(res.results)
    if _trace:
        return out, res
    return out


# revision 5
# speedup vs baseline: 1.2782x; 1.0035x over previous
"""GPTQ 4-bit quantized linear (column-parallel over 8 NeuronCores), v4.

y = x @ dequant(qweight, scales, zeros).T + bias with byte-packed 4-bit pairs.

Core trick: fp8e4m3 bit patterns 0..15 decode to exactly b * 2^-9 (denormals
plus the first normal binade are uniformly spaced), so nibbles extracted with
cheap fused DVE bitwise ops (and / shr+and on u16 lanes) can be BITCAST to
fp8e4 and fed to the PE against an fp16 x (mixed-dtype matmul): no cast
instructions and no fp8 x-split. The 2^9 is folded into the host scales.

Layout per core (out_features 11008 -> 8 x 1376):
  * k-tiles 0-13 ship packed (0.5 B/weight); the last pair (k14,15) ships as
    host-unpacked raw nibble bytes (same fp8-bitcast encoding, 2x bytes) so
    the tail of the stream needs no decode work at all.
  * Pool (gpsimd) absorbs the hi-nibble masks of a few early pairs.
  * D[o, t, c, b] per-group dots accumulate in PSUM; fix pieces [8,4,2,2]
    k-tiles: ACT copies PSUM to SBUF f16 [p,t,b,c] (transposing copy), DVE
    multiplies by sfix in 2x mode (b-broadcast, c innermost), halving TT-tree
    reduces over c, two f16 accumulator adds per piece.
  * Output: yt [128, 88] f16, one straight-copy DMA; host de-tiles.
"""

import numpy as np

import concourse.bacc as bacc
import concourse.mybir as mybir
import concourse.tile as tile
from concourse.bass_utils import run_bass_kernel_spmd

dt = mybir.dt
Alu = mybir.AluOpType

B = 8
I = 4096
O = 11008
NCORES = 8
OSH = O // NCORES            # 1376
OT = 11
OPW = OSH                    # 1376
NG = 32
NJ = 16                      # k-tiles (2048 / 128)

NPK = 12                     # packed k-tiles; k12-k15 ship as f8 bytes
PIECES = [8, 2, 2, 2, 2]     # k-tiles per fix piece (2 groups per tile)
# hi-mask engine per packed pair: "v" = DVE, "p" = Pool (gpsimd)
HI_ROUTES = ["v", "v", "v", "v", "v", "v"]

_piece_of = []
_slot_of = []
for _pi, _n in enumerate(PIECES):
    for _j in range(_n):
        _piece_of.append(_pi)
        _slot_of.append(2 * _j)

_nc_cache = None


def _build_nc():
    nc = bacc.Bacc("TRN2", target_bir_lowering=False)

    wq_d = {k0: nc.dram_tensor(f"wq{k0}", [2, 128, OPW], dt.uint8,
                               kind="ExternalInput") for k0 in range(0, NPK, 2)}
    f8_d = {"f8t12": nc.dram_tensor("f8t12", [128, 4 * OPW], dt.uint8,
                                    kind="ExternalInput"),
            "f8t14": nc.dram_tensor("f8t14", [128, 2 * OPW], dt.uint8,
                                    kind="ExternalInput"),
            "f8t15": nc.dram_tensor("f8t15", [128, 2 * OPW], dt.uint8,
                                    kind="ExternalInput")}  # k14/k15 split
    cblob_d = nc.dram_tensor("cblob", [128, 1392], dt.uint8, kind="ExternalInput")
    out = nc.dram_tensor("out", [128, OT * B], dt.float16, kind="ExternalOutput")

    with tile.TileContext(nc) as tc:
        with (
            tc.tile_pool(name="const", bufs=1) as constp,
            tc.tile_pool(name="wqp", bufs=4) as wqp,
            tc.tile_pool(name="nibp", bufs=6) as nibp,
            tc.tile_pool(name="fixp", bufs=1) as fixp,
            tc.tile_pool(name="dpsp", bufs=1, space="PSUM") as dpsp,
        ):
            cblob = constp.tile([128, 1392], dt.uint8)
            wq_sb = {}

            def wq_dma(k0):
                wq_sb[k0] = wqp.tile([128, 2, OPW], dt.uint8, name=f"wq{k0}")
                nc.sync.dma_start(wq_sb[k0][:],
                                  wq_d[k0][:].rearrange("n p o -> p n o"))

            # stream: wq0, cblob, wq2..wq10, f8 tiles last
            wq_dma(0)
            nc.sync.dma_start(cblob[:], cblob_d[:])
            xt_sb = cblob[:, 0:512].bitcast(dt.float16)          # [128, 32*8]
            sfix_sb = cblob[:, 512:1216].bitcast(dt.float16)     # [128, 11*32]
            corrb_sb = cblob[:, 1216:1392].bitcast(dt.float16)   # [128, 11*8]
            for k0 in (2, 4, 6, 8, 10):
                wq_dma(k0)
            f8_sb = {}
            f8_sb["f8t12"] = constp.tile([128, 4 * OPW], dt.uint8, name="f8t12")
            nc.sync.dma_start(f8_sb["f8t12"][:], f8_d["f8t12"][:])
            for nm in ("f8t14", "f8t15"):
                f8_sb[nm] = constp.tile([128, 2 * OPW], dt.uint8, name=nm)
                nc.sync.dma_start(f8_sb[nm][:], f8_d[nm][:])

            # PSUM: piece tiles (3+2+1+1+1 banks)
            d_ps = [dpsp.tile([128, OT, 2 * n, 8], dt.float32, name=f"d{i}",
                              tag=f"d{i}") for i, n in enumerate(PIECES)]

            def decode_pair(k0):
                src16 = wq_sb[k0][:].rearrange("p n o -> p (n o)").bitcast(dt.uint16)
                ehi = nc.gpsimd if HI_ROUTES[k0 // 2] == "p" else nc.vector
                lo8 = nibp.tile([128, 2, OPW], dt.uint8, tag="lo8", name=f"lo8_{k0}")
                nc.vector.tensor_scalar(
                    lo8[:].rearrange("p n o -> p (n o)").bitcast(dt.uint16),
                    src16, 0x0F0F, None, op0=Alu.bitwise_and)
                hi8 = nibp.tile([128, 2, OPW], dt.uint8, tag="hi8", name=f"hi8_{k0}")
                ehi.tensor_scalar(
                    hi8[:].rearrange("p n o -> p (n o)").bitcast(dt.uint16),
                    src16, 4, 0x0F0F,
                    op0=Alu.logical_shift_right, op1=Alu.bitwise_and)
                return lo8, hi8

            def mms(k, lo_ap, hi_ap):
                # lo_ap/hi_ap: [128, OPW] uint8 views holding nibble values
                dtile = d_ps[_piece_of[k]]
                clo = _slot_of[k]
                glo, ghi = k, k + 16
                for t in range(OT):
                    w = 128 if t < OT - 1 else OPW - 128 * (OT - 1)
                    dv = dtile[: w, t]
                    nc.tensor.matmul(dv[:, clo],
                                     lo_ap[:, t * 128:t * 128 + w].bitcast(dt.float8e4),
                                     xt_sb[:, glo * B:(glo + 1) * B],
                                     start=True, stop=True)
                    nc.tensor.matmul(dv[:, clo + 1],
                                     hi_ap[:, t * 128:t * 128 + w].bitcast(dt.float8e4),
                                     xt_sb[:, ghi * B:(ghi + 1) * B],
                                     start=True, stop=True)

            uA = fixp.tile([128, OT * B], dt.float16, name="uA")
            uB = fixp.tile([128, OT * B], dt.float16, name="uB")
            yt = fixp.tile([128, OT * B], dt.float16, name="yt")

            def fix_piece(pi):
                veng = nc.gpsimd if pi == 2 else nc.vector
                ncols = 2 * PIECES[pi]
                c0 = sum(2 * n for n in PIECES[:pi])
                dv = d_ps[pi][:]
                sf = sfix_sb.rearrange("p (t c) -> p t c", c=NG)[
                    :, :, c0:c0 + ncols]
                tmp = fixp.tile([128, OT, B, ncols], dt.float16,
                                tag=f"tmp{pi}", name=f"tmp{pi}")
                if pi == len(PIECES) - 1:
                    # last piece: multiply straight from PSUM on DVE (1x but
                    # small; skips the ACT hop on the critical tail). GPSIMD
                    # cannot read PSUM, so only DVE pieces may do this.
                    nc.vector.tensor_tensor(
                        tmp[:], dv.transpose([0, 1, 3, 2]),
                        sf.unsqueeze(2).broadcast_to([128, OT, B, ncols]),
                        Alu.mult)
                else:
                    # ACT: PSUM f32 [p,t,c,b] -> SBUF f16 [p,t,b,c]
                    cp = fixp.tile([128, OT, B, ncols], dt.float16,
                                   tag=f"cp{pi}", name=f"cp{pi}")
                    nc.scalar.copy(cp[:].transpose([0, 1, 3, 2]), dv)
                    veng.tensor_tensor(
                        tmp[:], cp[:],
                        sf.unsqueeze(2).broadcast_to([128, OT, B, ncols]),
                        Alu.mult)
                # halve over c down to 2 columns (2x TT tree), then fold the
                # piece into one of two alternating accumulator chains
                with nc.allow_low_precision(reason="f16 partial sums"):
                    w = ncols
                    while w > 2:
                        h = w // 2
                        veng.tensor_tensor(
                            tmp[:, :, :, 0:h], tmp[:, :, :, 0:h],
                            tmp[:, :, :, h:w], Alu.add)
                        w = h
                    red = fixp.tile([128, OT, B], dt.float16, tag=f"red{pi}",
                                    name=f"red{pi}")
                    veng.tensor_tensor(red[:], tmp[:, :, :, 0],
                                       tmp[:, :, :, 1], Alu.add)
                    rv = red[:].rearrange("p t b -> p (t b)")
                    npc = len(PIECES)
                    if pi == 0:
                        nc.vector.tensor_tensor(uA[:], corrb_sb, rv, Alu.add)
                    elif pi == 1:
                        nc.vector.tensor_copy(uB[:], rv)
                    elif pi == npc - 1:
                        # last piece: yt = uAB + red (uAB folded in at npc-2)
                        nc.vector.tensor_tensor(yt[:], uA[:], rv, Alu.add)
                    elif pi == npc - 2:
                        # fold uB on the idle Pool engine, off the DVE tail
                        nc.gpsimd.tensor_tensor(uB[:], uB[:], rv, Alu.add)
                        nc.vector.tensor_tensor(uA[:], uA[:], uB[:], Alu.add)
                    else:
                        eng = uA if pi % 2 == 0 else uB
                        nc.vector.tensor_tensor(eng[:], eng[:], rv, Alu.add)

            npieces = len(PIECES)
            fix_after = {}
            acc = 0
            for pi, n in enumerate(PIECES):
                acc += n
                fix_after[acc - 1] = pi

            for k0 in range(0, NPK, 2):
                lo8, hi8 = decode_pair(k0)
                for j in (0, 1):
                    k = k0 + j
                    mms(k, lo8[:, j], hi8[:, j])
                    if k in fix_after:
                        fix_piece(fix_after[k])
            # f8 tiles k12-k15 (nibble bytes: [lo hi] per tile)
            for j, k in enumerate((12, 13)):
                mms(k, f8_sb["f8t12"][:, (2 * j) * OPW:(2 * j + 1) * OPW],
                    f8_sb["f8t12"][:, (2 * j + 1) * OPW:(2 * j + 2) * OPW])
                if k in fix_after:
                    fix_piece(fix_after[k])
            for k in (14, 15):
                t = f8_sb[f"f8t{k}"]
                mms(k, t[:, 0:OPW], t[:, OPW:2 * OPW])
                if k in fix_after:
                    fix_piece(fix_after[k])

            nc.sync.dma_start(out[:], yt[:])

    nc.compile()
    return nc


def _get_nc():
    global _nc_cache
    if _nc_cache is None:
        _nc_cache = _build_nc()
    return _nc_cache


def _prep_inputs(x, qweight, scales, zeros, bias):
    x = np.asarray(x)
    qweight = np.asarray(qweight)
    scales = np.asarray(scales)
    zeros = np.asarray(zeros)
    bias = np.asarray(bias)

    qb = qweight.astype(np.uint8)
    nib = np.empty((O, I), np.uint8)
    nib[:, 0::2] = qb & 15
    nib[:, 1::2] = qb >> 4
    nibT = np.ascontiguousarray(nib.T)                    # (4096, 11008)
    wq_packed = nibT[: I // 2] | (nibT[I // 2:] << 4)     # (2048, 11008)

    xt_host = np.ascontiguousarray(
        x.T.reshape(NG, 128, B).transpose(1, 0, 2).reshape(128, NG * B)
    ).astype(np.float16)

    Xg = x.astype(np.float64).T.reshape(NG, 128, B).sum(axis=1)   # (32, 8)

    piece_base = []
    acc = 0
    for n in PIECES:
        piece_base.append(acc)
        acc += 2 * n
    col_lo = {}
    for k in range(NJ):
        col_lo[k] = piece_base[_piece_of[k]] + _slot_of[k]

    in_maps = []
    for cidx in range(NCORES):
        sl = slice(cidx * OSH, (cidx + 1) * OSH)
        s_c = scales[sl].astype(np.float64)      # (1376, 32)
        z_c = zeros[sl].astype(np.float64)
        b_c = bias[sl].astype(np.float64)
        wq_c = np.ascontiguousarray(wq_packed[:, sl])

        im = {}
        kt = wq_c.reshape(NJ, 128, OPW)
        for k0 in range(0, NPK, 2):
            im[f"wq{k0}"] = np.ascontiguousarray(kt[k0:k0 + 2])
        im["f8t12"] = np.ascontiguousarray(np.concatenate(
            [kt[12] & 15, kt[12] >> 4, kt[13] & 15, kt[13] >> 4], axis=1))
        for k in (14, 15):
            im[f"f8t{k}"] = np.ascontiguousarray(np.concatenate(
                [kt[k] & 15, kt[k] >> 4], axis=1))

        # sfix: value = s * 512 (fp8 bitcast carries 2^-9)
        sfix = np.zeros((OT * 128, NG), np.float64)
        for k in range(NJ):
            sfix[:OSH, col_lo[k]] = s_c[:, k] * 512.0
            sfix[:OSH, col_lo[k] + 1] = s_c[:, k + 16] * 512.0
        sfix_t = np.ascontiguousarray(
            sfix.reshape(OT, 128, NG).transpose(1, 0, 2).reshape(128, OT * NG)
        ).astype(np.float16)

        corr = (s_c * z_c) @ Xg                  # (1376, 8)
        corrb = np.zeros((OT * 128, B), np.float64)
        corrb[:OSH] = b_c[:, None] - corr
        corrb_t = np.ascontiguousarray(
            corrb.reshape(OT, 128, B).transpose(1, 0, 2).reshape(128, OT * B)
        ).astype(np.float16)

        cblob = np.concatenate([
            xt_host.view(np.uint8),
            sfix_t.view(np.uint8),
            corrb_t.view(np.uint8),
        ], axis=1)
        assert cblob.shape == (128, 1392), cblob.shape
        im["cblob"] = np.ascontiguousarray(cblob)
        in_maps.append(im)
    return in_maps


def _gather(results):
    outs = []
    for r in results:
        o = r["out"]                              # (128, 88) f16
        y = o.reshape(128, OT, B).transpose(1, 0, 2).reshape(OT * 128, B)
        outs.append(y[:OSH])
    y = np.concatenate(outs, axis=0)
    return np.ascontiguousarray(y.T)


def kernel(x, qweight, scales, zeros, bias, _trace=False):
    nc = _get_nc()
    in_maps = _prep_inputs(x, qweight, scales, zeros, bias)
    res = run_bass_kernel_spmd(
        nc, in_maps, core_ids=list(range(NCORES)), trace=_trace
    )
    out = _gather(res.results)
    if _trace:
        return out, res
    return out
